# revision 1
# baseline (speedup 1.0000x reference)
"""GNN message passing (scatter-add + relu) on 8 trn2 NeuronCores.

out = relu(segment_sum(x[src_all], dst_all)) with self-loops appended,
N=100000 nodes, E=1.6M edges, F=128 features.

Design (per core, SPMD over 8 cores, dst-shard partitioning):
  - core owns dst rows [core*12500, (core+1)*12500)
  - self-loop contribution = accumulator initialized from x's shard rows
  - edge tokens sorted into (src-chunk, dst-block) cells, each cell padded to
    a static capacity R so the device program is data-independent
  - dma_gather (GPSIMD SWDGE, single_packet=False) fetches x[src] rows (512B)
    from the core's full x replica in HBM, B=4096 tokens per call
  - one-hot selection built on DVE (is_equal vs iota); TensorE matmul scatters
    each 128-token group into the PSUM tile of its dst block (fp32, exact);
    PSUM drained by DVE add into an SBUF accumulator
  - relu on the accumulator, DMA out per block
Host gathers the 8 shards and concatenates.
"""

import numpy as np

N = 100000
F = 128
NCORES = 8
SHARD = N // NCORES          # 12500
NBLK = (SHARD + 127) // 128  # 98 blocks of 128 dst rows (last partial: 84)
NCHUNK = 4
CHS = N // NCHUNK            # 25000 rows per src chunk (int16-indexable)
OUT_ROWS = NBLK * 128        # 12544
B = 4096                     # tokens per dma_gather call

_PROGRAM_CACHE = {}
_TRACE = False               # set by test harness to capture HW exec time
_LAST_EXEC_NS = None
_LAST_RESULTS = None


def _plan_dims(R):
    nbatch_c = -(-NBLK * R // B)         # batches per chunk region
    ncalls = NCHUNK * nbatch_c
    return nbatch_c, ncalls


def _build_program(R):
    import concourse.tile as tile
    from concourse import bacc, mybir
    from contextlib import ExitStack

    G = R // 128                          # groups per cell
    gpb = B // 128                        # groups per batch (32)
    nbatch_c, ncalls = _plan_dims(R)
    real_groups_c = NBLK * G              # real groups per chunk region

    nc = bacc.Bacc("TRN2", num_devices=NCORES, debug=False,
                   num_swdge_queues=4)
    x_t = nc.dram_tensor("x", [N, F], mybir.dt.float32, kind="ExternalInput")
    g_t = nc.dram_tensor("gidx", [ncalls, 128, B // 16], mybir.dt.int16,
                         kind="ExternalInput")
    d_t = nc.dram_tensor("dloc", [ncalls, 128, gpb], mybir.dt.float32,
                         kind="ExternalInput")
    iota_t = nc.dram_tensor("iota", [128, gpb * 128], mybir.dt.float32,
                            kind="ExternalInput")
    out_t = nc.dram_tensor("out", [OUT_ROWS, F], mybir.dt.float32,
                           kind="ExternalOutput")
    xshard_t = nc.dram_tensor("xshard", [SHARD, F], mybir.dt.float32,
                              kind="ExternalInput")  # this core's shard rows

    with tile.TileContext(nc) as tc:
        with ExitStack() as ctx:
            const = ctx.enter_context(tc.tile_pool(name="const", bufs=1))
            accp = ctx.enter_context(tc.tile_pool(name="accp", bufs=1))
            gp = ctx.enter_context(tc.tile_pool(name="gp", bufs=4))
            dp = ctx.enter_context(tc.tile_pool(name="dp", bufs=4))
            featp = ctx.enter_context(tc.tile_pool(name="featp", bufs=4))
            selp = ctx.enter_context(tc.tile_pool(name="selp", bufs=4))
            psump = ctx.enter_context(tc.tile_pool(name="psump", bufs=8,
                                                   space="PSUM"))

            iota_sb = const.tile([128, gpb, 128], mybir.dt.float32)
            nc.sync.dma_start(iota_sb[:], iota_t[:].rearrange(
                "p (g f) -> p g f", g=gpb))

            # accumulator init = self-loop contribution x[shard rows]
            acc = accp.tile([128, NBLK, 128], mybir.dt.float32)
            nc.sync.dma_start(
                acc[:, :NBLK - 1, :],
                xshard_t[:(NBLK - 1) * 128, :].rearrange(
                    "(b p) f -> p b f", p=128),
            )
            last = SHARD - (NBLK - 1) * 128  # 84
            nc.vector.memset(acc[:, NBLK - 1, :], 0.0)
            nc.sync.dma_start(
                acc[:last, NBLK - 1, :],
                xshard_t[(NBLK - 1) * 128:, :],
            )

            for chunk in range(NCHUNK):
                psum_cur = None
                for jb in range(nbatch_c):
                    j = chunk * nbatch_c + jb
                    g_sb = gp.tile([128, B // 16], mybir.dt.int16, tag="g")
                    nc.sync.dma_start(g_sb[:], g_t[j])
                    d_sb = dp.tile([128, gpb], mybir.dt.float32, tag="d")
                    nc.sync.dma_start(d_sb[:], d_t[j])

                    feat = featp.tile([128, gpb, F], mybir.dt.float32, tag="f")
                    nc.gpsimd.dma_gather(
                        out_ap=feat[:],
                        in_ap=x_t[chunk * CHS:(chunk + 1) * CHS, :],
                        idxs_ap=g_sb[:],
                        num_idxs=B,
                        num_idxs_reg=B,
                        elem_size=F,
                        single_packet=False,
                        queue_num=j % 4,
                    )
                    sel = selp.tile([128, gpb, F], mybir.dt.float32, tag="s")
                    nc.vector.tensor_tensor(
                        out=sel[:],
                        in0=d_sb[:].unsqueeze(2).to_broadcast([128, gpb, F]),
                        in1=iota_sb[:],
                        op=mybir.AluOpType.is_equal,
                    )
                    for col in range(gpb):
                        gi = jb * gpb + col          # chunk-local group index
                        if gi >= real_groups_c:
                            break                    # chunk-tail pad groups
                        blk = gi // G
                        gph = gi % G                 # phase within cell
                        if gph == 0:
                            psum_cur = psump.tile([128, F], mybir.dt.float32,
                                                  tag="ps")
                        nc.tensor.matmul(
                            out=psum_cur[:],
                            lhsT=sel[:, col, :],
                            rhs=feat[:, col, :],
                            start=(gph == 0),
                            stop=(gph == G - 1),
                        )
                        if gph == G - 1:
                            dst = acc[:, blk, :]
                            nc.vector.tensor_add(out=dst, in0=dst,
                                                 in1=psum_cur[:])

            nc.vector.tensor_scalar_max(acc[:], acc[:], 0.0)
            for blk in range(NBLK):
                nc.sync.dma_start(out_t[blk * 128:(blk + 1) * 128, :],
                                  acc[:, blk, :])
    nc.compile()
    return nc


def _wrap16(tok):
    """[B] int16 token list -> [128, B//16] SWDGE idx layout (replicated x8)."""
    w = tok.reshape(-1, 16).T
    return np.tile(w, (8, 1))


def _prep_core(src, dst, R):
    """gidx/dloc planes for one core; src global [0,N), dst shard-local."""
    nbatch_c, ncalls = _plan_dims(R)
    gpb = B // 128
    slots_c = nbatch_c * B               # slot region per chunk

    chunk = src // CHS
    blk = dst // 128
    cell = chunk * NBLK + blk
    order = np.lexsort((src, cell))
    cell_s = cell[order]
    src_s = src[order]
    dst_s = dst[order]

    tot = NCHUNK * slots_c
    gidx = np.zeros(tot, dtype=np.int16)            # pad: row 0 of chunk
    dloc = np.full(tot, 200.0, dtype=np.float32)    # pad: no one-hot row
    counts = np.bincount(cell_s, minlength=NCHUNK * NBLK)
    starts = np.zeros_like(counts)
    np.cumsum(counts[:-1], out=starts[1:])
    # slot of cell c starts at (chunk of c)*slots_c + (blk of c)*R
    cell_base = (cell_s // NBLK) * slots_c + (cell_s % NBLK).astype(np.int64) * R
    slot = cell_base + (np.arange(len(cell_s)) - starts[cell_s])
    gidx[slot] = (src_s - (src_s // CHS) * CHS).astype(np.int16)
    dloc[slot] = (dst_s - (dst_s // 128) * 128).astype(np.float32)

    g_plane = np.zeros((ncalls, 128, B // 16), dtype=np.int16)
    d_plane = np.zeros((ncalls, 128, gpb), dtype=np.float32)
    for j in range(ncalls):
        seg = slice(j * B, (j + 1) * B)
        g_plane[j] = _wrap16(gidx[seg])
        d_plane[j] = dloc[seg].reshape(gpb, 128).T
    return g_plane, d_plane


def kernel(x, edge_index):
    from concourse import bass_utils

    x = np.ascontiguousarray(np.asarray(x, dtype=np.float32))
    ei = np.asarray(edge_index)
    src = ei[0].astype(np.int64)
    dst = ei[1].astype(np.int64)

    owner = dst // SHARD
    cell_id = (owner * NCHUNK + src // CHS) * NBLK + (dst % SHARD) // 128
    max_cnt = np.bincount(cell_id, minlength=NCORES * NCHUNK * NBLK).max()
    R = max(640, int(-(-max_cnt // 128) * 128))

    if R not in _PROGRAM_CACHE:
        _PROGRAM_CACHE[R] = _build_program(R)
    nc = _PROGRAM_CACHE[R]

    iota = np.tile(np.arange(128, dtype=np.float32), (128, B // 128))
    in_maps = []
    for core in range(NCORES):
        m = owner == core
        g_plane, d_plane = _prep_core(src[m], dst[m] - core * SHARD, R)
        in_maps.append({
            "x": x,
            "xshard": np.ascontiguousarray(
                x[core * SHARD:(core + 1) * SHARD]),
            "gidx": g_plane,
            "dloc": d_plane,
            "iota": iota,
        })

    kwargs = {"trace": True} if _TRACE else {}
    res = bass_utils.run_bass_kernel_spmd(nc, in_maps,
                                          core_ids=list(range(NCORES)),
                                          **kwargs)
    global _LAST_EXEC_NS, _LAST_RESULTS
    _LAST_EXEC_NS = res.exec_time_ns
    _LAST_RESULTS = res
    out = np.concatenate(
        [res.results[c]["out"][:SHARD] for c in range(NCORES)], axis=0)
    return out.astype(np.float32)



# revision 2
# speedup vs baseline: 1.4214x; 1.4214x over previous
"""GNN message passing (scatter-add + relu) on 8 trn2 NeuronCores.

out = relu(segment_sum(x[src_all], dst_all)) with self-loops appended,
N=100000 nodes, E=1.6M edges, F=128 features.

Design (per core, SPMD over 8 cores, dst-shard partitioning):
  - core owns dst rows [core*12500, (core+1)*12500)
  - self-loop contribution = accumulator initialized (fp32) from x's shard rows
  - edge tokens sorted into (src-chunk, dst-block) cells, each cell padded to
    a static capacity R so the device program is data-independent
  - x is converted to bf16 on host; dma_gather (GPSIMD SWDGE) fetches 256B
    bf16 rows from the core's full bf16 x replica in HBM, B=4096 tokens/call,
    4 SWDGE queues round-robin (desc-gen runs concurrently across Q7 pairs)
  - one-hot selection built on DVE in bf16 (is_equal vs iota); TensorE bf16
    matmul scatters each 128-token group into the PSUM tile (fp32) of its dst
    block; PSUM drained by DVE add into the fp32 SBUF accumulator
  - relu on the accumulator, single batched DMA out
Host gathers the 8 shards and concatenates.
"""

import numpy as np

N = 100000
F = 128
NCORES = 8
SHARD = N // NCORES          # 12500
NBLK = (SHARD + 127) // 128  # 98 blocks of 128 dst rows (last partial: 84)
NCHUNK = 4
CHS = N // NCHUNK            # 25000 rows per src chunk (int16-indexable)
OUT_ROWS = NBLK * 128        # 12544
B = 4096                     # tokens per dma_gather call

_PROGRAM_CACHE = {}
_TRACE = False               # set by test harness to capture HW exec time
_LAST_EXEC_NS = None
_LAST_RESULTS = None


def _plan_dims(R):
    nbatch_c = -(-NBLK * R // B)         # batches per chunk region
    ncalls = NCHUNK * nbatch_c
    return nbatch_c, ncalls


def _build_program(R):
    import concourse.tile as tile
    from concourse import bacc, mybir
    from contextlib import ExitStack

    G = R // 128                          # groups per cell
    gpb = B // 128                        # groups per batch (32)
    nbatch_c, ncalls = _plan_dims(R)
    real_groups_c = NBLK * G              # real groups per chunk region

    nc = bacc.Bacc("TRN2", num_devices=NCORES, debug=False,
                   num_swdge_queues=4)
    xbf_t = nc.dram_tensor("xbf", [N, F], mybir.dt.bfloat16,
                           kind="ExternalInput")
    g_t = nc.dram_tensor("gidx", [ncalls, 128, B // 16], mybir.dt.int16,
                         kind="ExternalInput")
    d_t = nc.dram_tensor("dloc", [ncalls, 128, gpb], mybir.dt.bfloat16,
                         kind="ExternalInput")
    iota_t = nc.dram_tensor("iota", [128, gpb * 128], mybir.dt.bfloat16,
                            kind="ExternalInput")
    out_t = nc.dram_tensor("out", [OUT_ROWS, F], mybir.dt.float32,
                           kind="ExternalOutput")
    xshard_t = nc.dram_tensor("xshard", [SHARD, F], mybir.dt.float32,
                              kind="ExternalInput")  # this core's shard rows

    with tile.TileContext(nc) as tc:
        with ExitStack() as ctx:
            const = ctx.enter_context(tc.tile_pool(name="const", bufs=1))
            accp = ctx.enter_context(tc.tile_pool(name="accp", bufs=1))
            gp = ctx.enter_context(tc.tile_pool(name="gp", bufs=8))
            dp = ctx.enter_context(tc.tile_pool(name="dp", bufs=8))
            featp = ctx.enter_context(tc.tile_pool(name="featp", bufs=6))
            selp = ctx.enter_context(tc.tile_pool(name="selp", bufs=6))
            psump = ctx.enter_context(tc.tile_pool(name="psump", bufs=8,
                                                   space="PSUM"))

            iota_sb = const.tile([128, gpb, 128], mybir.dt.bfloat16)
            nc.sync.dma_start(iota_sb[:], iota_t[:].rearrange(
                "p (g f) -> p g f", g=gpb))

            # accumulator init = self-loop contribution x[shard rows]
            acc = accp.tile([128, NBLK, 128], mybir.dt.float32)
            nc.sync.dma_start(
                acc[:, :NBLK - 1, :],
                xshard_t[:(NBLK - 1) * 128, :].rearrange(
                    "(b p) f -> p b f", p=128),
            )
            last = SHARD - (NBLK - 1) * 128  # 84
            nc.vector.memset(acc[:, NBLK - 1, :], 0.0)
            nc.sync.dma_start(
                acc[:last, NBLK - 1, :],
                xshard_t[(NBLK - 1) * 128:, :],
            )

            for chunk in range(NCHUNK):
                psum_cur = None
                for jb in range(nbatch_c):
                    j = chunk * nbatch_c + jb
                    g_sb = gp.tile([128, B // 16], mybir.dt.int16, tag="g")
                    nc.sync.dma_start(g_sb[:], g_t[j])
                    d_sb = dp.tile([128, gpb], mybir.dt.bfloat16, tag="d")
                    nc.sync.dma_start(d_sb[:], d_t[j])

                    feat = featp.tile([128, gpb, F], mybir.dt.bfloat16,
                                      tag="f")
                    nc.gpsimd.dma_gather(
                        out_ap=feat[:],
                        in_ap=xbf_t[chunk * CHS:(chunk + 1) * CHS, :],
                        idxs_ap=g_sb[:],
                        num_idxs=B,
                        num_idxs_reg=B,
                        elem_size=F,
                        single_packet=False,
                        queue_num=j % 4,
                    )
                    sel = selp.tile([128, gpb, F], mybir.dt.bfloat16, tag="s")
                    nc.vector.tensor_tensor(
                        out=sel[:],
                        in0=d_sb[:].unsqueeze(2).to_broadcast([128, gpb, F]),
                        in1=iota_sb[:],
                        op=mybir.AluOpType.is_equal,
                    )
                    for col in range(gpb):
                        gi = jb * gpb + col          # chunk-local group index
                        if gi >= real_groups_c:
                            break                    # chunk-tail pad groups
                        blk = gi // G
                        gph = gi % G                 # phase within cell
                        if gph == 0:
                            psum_cur = psump.tile([128, F], mybir.dt.float32,
                                                  tag="ps")
                        nc.tensor.matmul(
                            out=psum_cur[:],
                            lhsT=sel[:, col, :],
                            rhs=feat[:, col, :],
                            start=(gph == 0),
                            stop=(gph == G - 1),
                        )
                        if gph == G - 1:
                            dst = acc[:, blk, :]
                            nc.vector.tensor_add(out=dst, in0=dst,
                                                 in1=psum_cur[:])

            nc.vector.tensor_scalar_max(acc[:], acc[:], 0.0)
            nc.sync.dma_start(
                out_t[:].rearrange("(b p) f -> p b f", p=128),
                acc[:],
            )
    nc.compile()
    return nc


def _wrap16(tok):
    """[B] int16 token list -> [128, B//16] SWDGE idx layout (replicated x8)."""
    w = tok.reshape(-1, 16).T
    return np.tile(w, (8, 1))


def _prep_core(src, dst, R):
    """gidx/dloc planes for one core; src global [0,N), dst shard-local."""
    import ml_dtypes

    nbatch_c, ncalls = _plan_dims(R)
    gpb = B // 128
    slots_c = nbatch_c * B               # slot region per chunk

    chunk = src // CHS
    blk = dst // 128
    cell = chunk * NBLK + blk
    order = np.lexsort((src, cell))
    cell_s = cell[order]
    src_s = src[order]
    dst_s = dst[order]

    tot = NCHUNK * slots_c
    gidx = np.zeros(tot, dtype=np.int16)            # pad: row 0 of chunk
    dloc = np.full(tot, 200.0, dtype=np.float32)    # pad: no one-hot row
    counts = np.bincount(cell_s, minlength=NCHUNK * NBLK)
    starts = np.zeros_like(counts)
    np.cumsum(counts[:-1], out=starts[1:])
    # slot of cell c starts at (chunk of c)*slots_c + (blk of c)*R
    cell_base = (cell_s // NBLK) * slots_c + (cell_s % NBLK).astype(np.int64) * R
    slot = cell_base + (np.arange(len(cell_s)) - starts[cell_s])
    gidx[slot] = (src_s - (src_s // CHS) * CHS).astype(np.int16)
    dloc[slot] = (dst_s - (dst_s // 128) * 128).astype(np.float32)

    g_plane = np.zeros((ncalls, 128, B // 16), dtype=np.int16)
    d_plane = np.zeros((ncalls, 128, gpb), dtype=ml_dtypes.bfloat16)
    for j in range(ncalls):
        seg = slice(j * B, (j + 1) * B)
        g_plane[j] = _wrap16(gidx[seg])
        d_plane[j] = dloc[seg].reshape(gpb, 128).T.astype(ml_dtypes.bfloat16)
    return g_plane, d_plane


def kernel(x, edge_index):
    import ml_dtypes
    from concourse import bass_utils

    x = np.ascontiguousarray(np.asarray(x, dtype=np.float32))
    xbf = np.ascontiguousarray(x.astype(ml_dtypes.bfloat16))
    ei = np.asarray(edge_index)
    src = ei[0].astype(np.int64)
    dst = ei[1].astype(np.int64)

    owner = dst // SHARD
    cell_id = (owner * NCHUNK + src // CHS) * NBLK + (dst % SHARD) // 128
    max_cnt = np.bincount(cell_id, minlength=NCORES * NCHUNK * NBLK).max()
    R = max(640, int(-(-max_cnt // 128) * 128))

    if R not in _PROGRAM_CACHE:
        _PROGRAM_CACHE[R] = _build_program(R)
    nc = _PROGRAM_CACHE[R]

    gpb = B // 128
    iota = np.tile(np.arange(128, dtype=np.float32),
                   (128, gpb)).astype(ml_dtypes.bfloat16)
    in_maps = []
    for core in range(NCORES):
        m = owner == core
        g_plane, d_plane = _prep_core(src[m], dst[m] - core * SHARD, R)
        in_maps.append({
            "xbf": xbf,
            "xshard": np.ascontiguousarray(
                x[core * SHARD:(core + 1) * SHARD]),
            "gidx": g_plane,
            "dloc": d_plane,
            "iota": iota,
        })

    kwargs = {"trace": True} if _TRACE else {}
    res = bass_utils.run_bass_kernel_spmd(nc, in_maps,
                                          core_ids=list(range(NCORES)),
                                          **kwargs)
    global _LAST_EXEC_NS, _LAST_RESULTS
    _LAST_EXEC_NS = res.exec_time_ns
    _LAST_RESULTS = res
    out = np.concatenate(
        [res.results[c]["out"][:SHARD] for c in range(NCORES)], axis=0)
    return out.astype(np.float32)


# revision 3
# speedup vs baseline: 1.6703x; 1.1751x over previous
"""GNN message passing (scatter-add + relu) on 8 trn2 NeuronCores.

out = relu(segment_sum(x[src_all], dst_all)) with self-loops appended,
N=100000 nodes, E=1.6M edges, F=128 features.

Design (per core, SPMD over 8 cores, dst-shard partitioning):
  - core owns dst rows [core*12500, (core+1)*12500)
  - self-loop contribution = accumulator initialized (fp32) from x's shard rows
  - edge tokens sorted into (src-chunk, dst-block) cells, each cell padded to
    a static capacity R so the device program is data-independent
  - x is converted to bf16 on host; dma_gather (GPSIMD SWDGE) fetches 256B
    bf16 rows from the core's full bf16 x replica in HBM, B=4096 tokens/call,
    4 SWDGE queues round-robin (desc-gen runs concurrently across Q7 pairs)
  - one-hot selection built on DVE in bf16 (is_equal vs iota); TensorE bf16
    matmul scatters each 128-token group into the PSUM tile (fp32) of its dst
    block; PSUM drained by DVE add into the fp32 SBUF accumulator
  - relu on the accumulator, single batched DMA out
Host gathers the 8 shards and concatenates.
"""

import numpy as np

N = 100000
F = 128
NCORES = 8
SHARD = N // NCORES          # 12500
NBLK = (SHARD + 127) // 128  # 98 blocks of 128 dst rows (last partial: 84)
NCHUNK = 4
CHS = N // NCHUNK            # 25000 rows per src chunk (int16-indexable)
OUT_ROWS = NBLK * 128        # 12544
B = 2048                     # tokens per dma_gather call

_PROGRAM_CACHE = {}
_TRACE = False               # set by test harness to capture HW exec time
_LAST_EXEC_NS = None
_LAST_RESULTS = None


def _plan_dims(R):
    nbatch_c = -(-NBLK * R // B)         # batches per chunk region
    ncalls = NCHUNK * nbatch_c
    return nbatch_c, ncalls


def _build_program(R):
    import concourse.tile as tile
    from concourse import bacc, mybir
    from contextlib import ExitStack

    G = R // 128                          # groups per cell
    gpb = B // 128                        # groups per batch (32)
    nbatch_c, ncalls = _plan_dims(R)
    real_groups_c = NBLK * G              # real groups per chunk region

    nc = bacc.Bacc("TRN2", num_devices=NCORES, debug=False,
                   num_swdge_queues=4)
    xbf_t = nc.dram_tensor("xbf", [N, F], mybir.dt.bfloat16,
                           kind="ExternalInput")
    g_t = nc.dram_tensor("gidx", [ncalls, 128, B // 16], mybir.dt.int16,
                         kind="ExternalInput")
    d_t = nc.dram_tensor("dloc", [ncalls, 128, gpb], mybir.dt.bfloat16,
                         kind="ExternalInput")
    iota_t = nc.dram_tensor("iota", [128, gpb * 128], mybir.dt.bfloat16,
                            kind="ExternalInput")
    out_t = nc.dram_tensor("out", [OUT_ROWS, F], mybir.dt.float32,
                           kind="ExternalOutput")
    xshard_t = nc.dram_tensor("xshard", [SHARD, F], mybir.dt.float32,
                              kind="ExternalInput")  # this core's shard rows

    with tile.TileContext(nc) as tc:
        with ExitStack() as ctx:
            const = ctx.enter_context(tc.tile_pool(name="const", bufs=1))
            accp = ctx.enter_context(tc.tile_pool(name="accp", bufs=1))
            gp = ctx.enter_context(tc.tile_pool(name="gp", bufs=8))
            dp = ctx.enter_context(tc.tile_pool(name="dp", bufs=8))
            featp = ctx.enter_context(tc.tile_pool(name="featp", bufs=8))
            selp = ctx.enter_context(tc.tile_pool(name="selp", bufs=8))
            psump = ctx.enter_context(tc.tile_pool(name="psump", bufs=8,
                                                   space="PSUM"))

            iota_sb = const.tile([128, gpb, 128], mybir.dt.bfloat16)
            nc.sync.dma_start(iota_sb[:], iota_t[:].rearrange(
                "p (g f) -> p g f", g=gpb))

            # accumulator init = self-loop contribution x[shard rows]
            acc = accp.tile([128, NBLK, 128], mybir.dt.float32)
            nc.sync.dma_start(
                acc[:, :NBLK - 1, :],
                xshard_t[:(NBLK - 1) * 128, :].rearrange(
                    "(b p) f -> p b f", p=128),
            )
            last = SHARD - (NBLK - 1) * 128  # 84
            nc.vector.memset(acc[:, NBLK - 1, :], 0.0)
            nc.sync.dma_start(
                acc[:last, NBLK - 1, :],
                xshard_t[(NBLK - 1) * 128:, :],
            )

            nidx_reg = nc.gpsimd.to_reg(B)
            for chunk in range(NCHUNK):
                psum_cur = None
                for jb in range(nbatch_c):
                    j = chunk * nbatch_c + jb
                    g_sb = gp.tile([128, B // 16], mybir.dt.int16, tag="g")
                    nc.sync.dma_start(g_sb[:], g_t[j])
                    d_sb = dp.tile([128, gpb], mybir.dt.bfloat16, tag="d")
                    nc.sync.dma_start(d_sb[:], d_t[j])

                    feat = featp.tile([128, gpb, F], mybir.dt.bfloat16,
                                      tag="f")
                    nc.gpsimd.dma_gather(
                        out_ap=feat[:],
                        in_ap=xbf_t[chunk * CHS:(chunk + 1) * CHS, :],
                        idxs_ap=g_sb[:],
                        num_idxs=B,
                        num_idxs_reg=nidx_reg,
                        elem_size=F,
                        single_packet=False,
                        queue_num=j % 4,
                    )
                    sel = selp.tile([128, gpb, F], mybir.dt.bfloat16, tag="s")
                    nc.vector.tensor_tensor(
                        out=sel[:],
                        in0=d_sb[:].unsqueeze(2).to_broadcast([128, gpb, F]),
                        in1=iota_sb[:],
                        op=mybir.AluOpType.is_equal,
                    )
                    for col in range(gpb):
                        gi = jb * gpb + col          # chunk-local group index
                        if gi >= real_groups_c:
                            break                    # chunk-tail pad groups
                        blk = gi // G
                        gph = gi % G                 # phase within cell
                        if gph == 0:
                            psum_cur = psump.tile([128, F], mybir.dt.float32,
                                                  tag="ps")
                        nc.tensor.matmul(
                            out=psum_cur[:],
                            lhsT=sel[:, col, :],
                            rhs=feat[:, col, :],
                            start=(gph == 0),
                            stop=(gph == G - 1),
                        )
                        if gph == G - 1:
                            dst = acc[:, blk, :]
                            nc.vector.tensor_add(out=dst, in0=dst,
                                                 in1=psum_cur[:])

            nc.vector.tensor_scalar_max(acc[:], acc[:], 0.0)
            nc.sync.dma_start(
                out_t[:].rearrange("(b p) f -> p b f", p=128),
                acc[:],
            )
    nc.compile()
    return nc


def _wrap16(tok):
    """[B] int16 token list -> [128, B//16] SWDGE idx layout (replicated x8)."""
    w = tok.reshape(-1, 16).T
    return np.tile(w, (8, 1))


def _prep_core(src, dst, R):
    """gidx/dloc planes for one core; src global [0,N), dst shard-local."""
    import ml_dtypes

    nbatch_c, ncalls = _plan_dims(R)
    gpb = B // 128
    slots_c = nbatch_c * B               # slot region per chunk

    chunk = src // CHS
    blk = dst // 128
    cell = chunk * NBLK + blk
    order = np.lexsort((src, cell))
    cell_s = cell[order]
    src_s = src[order]
    dst_s = dst[order]

    tot = NCHUNK * slots_c
    gidx = np.zeros(tot, dtype=np.int16)            # pad: row 0 of chunk
    dloc = np.full(tot, 200.0, dtype=np.float32)    # pad: no one-hot row
    counts = np.bincount(cell_s, minlength=NCHUNK * NBLK)
    starts = np.zeros_like(counts)
    np.cumsum(counts[:-1], out=starts[1:])
    # slot of cell c starts at (chunk of c)*slots_c + (blk of c)*R
    cell_base = (cell_s // NBLK) * slots_c + (cell_s % NBLK).astype(np.int64) * R
    slot = cell_base + (np.arange(len(cell_s)) - starts[cell_s])
    gidx[slot] = (src_s - (src_s // CHS) * CHS).astype(np.int16)
    dloc[slot] = (dst_s - (dst_s // 128) * 128).astype(np.float32)

    g_plane = np.zeros((ncalls, 128, B // 16), dtype=np.int16)
    d_plane = np.zeros((ncalls, 128, gpb), dtype=ml_dtypes.bfloat16)
    for j in range(ncalls):
        seg = slice(j * B, (j + 1) * B)
        g_plane[j] = _wrap16(gidx[seg])
        d_plane[j] = dloc[seg].reshape(gpb, 128).T.astype(ml_dtypes.bfloat16)
    return g_plane, d_plane


def kernel(x, edge_index):
    import ml_dtypes
    from concourse import bass_utils

    x = np.ascontiguousarray(np.asarray(x, dtype=np.float32))
    xbf = np.ascontiguousarray(x.astype(ml_dtypes.bfloat16))
    ei = np.asarray(edge_index)
    src = ei[0].astype(np.int64)
    dst = ei[1].astype(np.int64)

    owner = dst // SHARD
    cell_id = (owner * NCHUNK + src // CHS) * NBLK + (dst % SHARD) // 128
    max_cnt = np.bincount(cell_id, minlength=NCORES * NCHUNK * NBLK).max()
    R = max(640, int(-(-max_cnt // 128) * 128))

    if R not in _PROGRAM_CACHE:
        _PROGRAM_CACHE[R] = _build_program(R)
    nc = _PROGRAM_CACHE[R]

    gpb = B // 128
    iota = np.tile(np.arange(128, dtype=np.float32),
                   (128, gpb)).astype(ml_dtypes.bfloat16)
    in_maps = []
    for core in range(NCORES):
        m = owner == core
        g_plane, d_plane = _prep_core(src[m], dst[m] - core * SHARD, R)
        in_maps.append({
            "xbf": xbf,
            "xshard": np.ascontiguousarray(
                x[core * SHARD:(core + 1) * SHARD]),
            "gidx": g_plane,
            "dloc": d_plane,
            "iota": iota,
        })

    kwargs = {"trace": True} if _TRACE else {}
    res = bass_utils.run_bass_kernel_spmd(nc, in_maps,
                                          core_ids=list(range(NCORES)),
                                          **kwargs)
    global _LAST_EXEC_NS, _LAST_RESULTS
    _LAST_EXEC_NS = res.exec_time_ns
    _LAST_RESULTS = res
    out = np.concatenate(
        [res.results[c]["out"][:SHARD] for c in range(NCORES)], axis=0)
    return out.astype(np.float32)
